# revision 1
# baseline (speedup 1.0000x reference)
"""Multi-head causal attention (B=4,T=2048,C=1024,H=16,D=64) on 8 TRN2 NeuronCores.

Sharding: no collectives. Core c handles batch b=c//2 and a causally-balanced
set of four 256-query chunks (half=c%2): half0 -> chunks [7,5,2,0], half1 ->
[6,4,3,1]. Every core runs the same program with padded per-slot key-tile
counts [16,12,8,4]; per-core differences (real counts / causal diagonals) are
expressed purely through per-core input data (mask tiles). K/V projections are
computed for the full sequence on both cores of a batch (duplication instead
of cross-core communication, which is far slower on this part).

Pipeline per core (one Bass/Tile program):
  B-stage: qT = (x @ Wq)^T for the core's 1024 query columns -> SBUF resident.
  A-stage: kT = (x @ Wk)^T -> DRAM scratch; v = x @ Wv (+ones col) -> DRAM.
  C-stage: per head-pair, per key tile j: scores = kT_j^T @ qT (fp32r,
           2 heads packed via PE row groups, separate PSUM banks), exp on ACT
           (scale folded in), causal/pad masks multiplied on the one slot that
           needs them, wei @ [v|1] accumulated in PSUM ([65,256] per head:
           row 64 = sumexp). Normalize with DVE fast reciprocal + GPSIMD
           partition broadcast.
  proj:    out = attn^T-layout tiles @ Wp (+bias), written per chunk.
All matmuls run as float32r with moving free dim >= 256 (full PE rate).
"""

import numpy as np

import concourse.bass as bass
import concourse.tile as tile
from concourse import bacc, library_config, mybir
from concourse.bass_utils import run_bass_kernel_spmd

B, T, C = 4, 2048, 1024
H, D = 16, 64
P = 128            # key tile size
QC = 256           # query chunk size
NP = 8             # head pairs
PN = [16, 12, 8, 4]                    # padded per-slot key-tile counts
CHUNKS = [[7, 5, 2, 0], [6, 4, 3, 1]]  # chunk ids per half, slot order
F32 = mybir.dt.float32
F32R = mybir.dt.float32r
EXP = mybir.ActivationFunctionType.Exp
SCALE = float(C) ** -0.5


def r(ap):
    """fp32 AP -> fp32r view for full-rate PE matmuls."""
    return ap.bitcast(F32R)


def build_kernel(nc: bass.Bass):
    xT = nc.dram_tensor("xT", [C, T], F32, kind="ExternalInput").ap()
    xq = nc.dram_tensor("xq", [C, 4 * QC], F32, kind="ExternalInput").ap()
    wq2 = nc.dram_tensor("wq2", [C, C], F32, kind="ExternalInput").ap()
    wk2 = nc.dram_tensor("wk2", [C, C], F32, kind="ExternalInput").ap()
    wv2 = nc.dram_tensor("wv2", [C, C], F32, kind="ExternalInput").ap()
    wp = nc.dram_tensor("wp", [C, C], F32, kind="ExternalInput").ap()
    bias = nc.dram_tensor("bias", [1, C], F32, kind="ExternalInput").ap()
    masks = nc.dram_tensor("masks", [16, P, QC], F32, kind="ExternalInput").ap()
    eye = nc.dram_tensor("eye", [P, P], F32, kind="ExternalInput").ap()
    out = nc.dram_tensor("out", [4, QC, C], F32, kind="ExternalOutput").ap()

    kT_d = nc.dram_tensor("kT_scratch", [C, T], F32).ap()
    v_d = nc.dram_tensor("v_scratch", [16, P, 16 * 65], F32).ap()

    with tile.TileContext(nc) as tc:
        nc.gpsimd.load_library(library_config.attn)
        with tc.tile_pool(name="const", bufs=1) as cpool:
            qT_sb = cpool.tile([P, NP * 1024], F32R)
            attn_sb = cpool.tile([P, NP * 1024], F32R)
            masks_sb = cpool.tile([P, 16 * QC], F32R)
            eye_sb = cpool.tile([P, P], F32R)

            # ---------------- B then A projections ----------------
            with (
                tc.tile_pool(name="ps_mm", bufs=2, space="PSUM") as ps_mm,
                tc.tile_pool(name="xt", bufs=8) as xtpool,
                tc.tile_pool(name="stage", bufs=2) as stpool,
                tc.tile_pool(name="vstage", bufs=1) as vstpool,
            ):
                with tc.tile_pool(name="wqp", bufs=1) as wqpool:
                    wq_sb = wqpool.tile([P, 8 * 1024], F32R, tag="wq")
                    for g in range(8):
                        nc.sync.dma_start(
                            wq_sb[:, g * 1024:(g + 1) * 1024],
                            wq2[g * P:(g + 1) * P, :].bitcast(F32R),
                        )
                    for k in range(4):
                        xqg = [
                            xtpool.tile([P, QC], F32R, tag="xt0", name=f"xq{g}")
                            for g in range(8)
                        ]
                        for g in range(8):
                            nc.sync.dma_start(
                                xqg[g][:],
                                xq[g * P:(g + 1) * P,
                                   k * QC:(k + 1) * QC].bitcast(F32R),
                            )
                        for p in range(NP):
                            qps = ps_mm.tile([P, QC], F32, tag="mm")
                            for g in range(8):
                                nc.tensor.matmul(
                                    qps[:],
                                    r(wq_sb[:, g * 1024 + (2 * p) * 64:][:, :128]),
                                    xqg[g][:],
                                    start=(g == 0), stop=(g == 7),
                                )
                            nc.scalar.copy(
                                qT_sb[:, p * 1024 + k * QC:][:, :QC], qps[:]
                            )

                with tc.tile_pool(name="wkvp", bufs=1) as wkvpool:
                    wk_sb = wkvpool.tile([P, 8 * 1024], F32R, tag="wk")
                    wv_sb = wkvpool.tile([P, 8 * 1024], F32R, tag="wv")
                    for g in range(8):
                        gs = slice(g * 1024, (g + 1) * 1024)
                        nc.sync.dma_start(
                            wk_sb[:, gs], wk2[g * P:(g + 1) * P, :].bitcast(F32R)
                        )
                        nc.sync.dma_start(
                            wv_sb[:, gs], wv2[g * P:(g + 1) * P, :].bitcast(F32R)
                        )
                    for tb in range(4):
                        ts_ = slice(tb * 512, (tb + 1) * 512)
                        xtg = [
                            xtpool.tile([P, 512], F32R, tag=f"xt{tb % 2}",
                                        name=f"xt{g}")
                            for g in range(8)
                        ]
                        for g in range(8):
                            nc.sync.dma_start(
                                xtg[g][:], xT[g * P:(g + 1) * P, ts_].bitcast(F32R)
                            )
                        for p in range(NP):
                            kps = ps_mm.tile([P, 512], F32, tag="mm")
                            for g in range(8):
                                nc.tensor.matmul(
                                    kps[:],
                                    r(wk_sb[:, g * 1024 + (2 * p) * 64:][:, :128]),
                                    xtg[g][:],
                                    start=(g == 0), stop=(g == 7),
                                )
                            kst = stpool.tile([P, 512], F32, tag="kst")
                            nc.vector.tensor_copy(kst[:], kps[:])
                            nc.sync.dma_start(kT_d[p * P:(p + 1) * P, ts_], kst[:])
                        for sti in range(4):
                            j = tb * 4 + sti
                            vst = vstpool.tile([P, 16 * 65], F32, tag="vst")
                            vv = vst[:].rearrange("p (h e) -> p h e", e=65)
                            nc.vector.memset(vv[:, :, 64:65], 1.0)
                            for hc in range(2):
                                vps = ps_mm.tile([P, 512], F32, tag="mm")
                                for g in range(8):
                                    nc.tensor.matmul(
                                        vps[:],
                                        r(xtg[g][:, sti * P:(sti + 1) * P]),
                                        wv_sb[:, g * 1024 + hc * 512:][:, :512],
                                        start=(g == 0), stop=(g == 7),
                                    )
                                nc.scalar.copy(
                                    vv[:, hc * 8:(hc + 1) * 8, 0:64],
                                    vps[:].rearrange("p (h d) -> p h d", d=64),
                                )
                            nc.sync.dma_start(v_d[j], vst[:])

            # ---------------- C: attention + proj ----------------
            for i in range(16):
                nc.sync.dma_start(
                    masks_sb[:, i * QC:(i + 1) * QC], masks[i].bitcast(F32R)
                )
            nc.sync.dma_start(eye_sb[:], eye[:].bitcast(F32R))
            with (
                tc.tile_pool(name="kv", bufs=2) as kvpool,
                tc.tile_pool(name="exp", bufs=3) as epool,
                tc.tile_pool(name="norm", bufs=1) as npool,
                tc.tile_pool(name="wpp", bufs=1) as wppool,
                tc.tile_pool(name="outp", bufs=3) as outpool,
                tc.tile_pool(name="ps_sc", bufs=2, space="PSUM") as ps_sc,
                tc.tile_pool(name="ps_av", bufs=2, space="PSUM") as ps_av,
                tc.tile_pool(name="ps_pj", bufs=2, space="PSUM") as ps_pj,
            ):
                wp_sb = wppool.tile([P, 8 * 1024], F32R, tag="wp")
                for g in range(8):
                    nc.sync.dma_start(
                        wp_sb[:, g * 1024:(g + 1) * 1024],
                        wp[g * P:(g + 1) * P, :].bitcast(F32R),
                    )
                bias_s = wppool.tile([1, C], F32, tag="bias1")
                nc.sync.dma_start(bias_s[:], bias[:])
                bias_bc = wppool.tile([P, C], F32, tag="biasbc")
                nc.gpsimd.partition_broadcast(bias_bc[:], bias_s[:])

                def c_run(k, p):
                    avp = ps_av.tile([65, 2 * QC], F32, tag="av",
                                     name=f"av{k}_{p}")
                    qA = qT_sb[0:64, p * 1024 + k * QC:][:, :QC]
                    qB = qT_sb[64:128, p * 1024 + k * QC:][:, :QC]
                    njc = PN[k] // 4
                    pend = None  # (e_t, v0, v1, j0) awaiting AV emission

                    def emit_av(pv):
                        e_t, v0, v1, j0 = pv
                        nc.tensor.matmul(avp[:, 0:QC], v0[:, 0:65],
                                         e_t[:, 0:QC],
                                         start=(j0 == 0), stop=False)
                        nc.tensor.matmul(avp[:, 0:QC], v1[:, 0:65],
                                         e_t[:, QC:2 * QC],
                                         start=False, stop=False)
                        nc.tensor.matmul(avp[:, QC:2 * QC], v0[:, 65:130],
                                         e_t[:, 2 * QC:3 * QC],
                                         start=False, stop=False)
                        nc.tensor.matmul(avp[:, QC:2 * QC], v1[:, 65:130],
                                         e_t[:, 3 * QC:4 * QC],
                                         start=False, stop=(j0 + 1 == PN[k] - 1))

                    for jc in range(njc):
                        ktc = kvpool.tile([P, 4 * P], F32R, tag="kt")
                        nc.sync.dma_start(
                            ktc[:],
                            kT_d[p * P:(p + 1) * P,
                                 jc * 4 * P:(jc + 1) * 4 * P].bitcast(F32R),
                        )
                        vc = kvpool.tile([P, 4 * 130], F32R, tag="vt")
                        nc.sync.dma_start(
                            vc[:].rearrange("s (j c) -> s j c", c=130),
                            v_d[4 * jc:4 * jc + 4, :,
                                2 * p * 65:(2 * p + 2) * 65]
                            .rearrange("j s c -> s j c").bitcast(F32R),
                        )
                        for u in range(2):
                            j0 = 4 * jc + 2 * u
                            masked = j0 >= PN[k] - 4
                            kt0 = ktc[:, (2 * u) * P:(2 * u + 1) * P]
                            kt1 = ktc[:, (2 * u + 1) * P:(2 * u + 2) * P]
                            v0 = vc[:, (2 * u) * 130:(2 * u + 1) * 130]
                            v1 = vc[:, (2 * u + 1) * 130:(2 * u + 2) * 130]
                            sc = ps_sc.tile([P, 4 * QC], F32, tag="sc")
                            nc.tensor.matmul(sc[:, 0:QC], r(kt0[0:64, :]), qA,
                                             start=True, stop=False,
                                             tile_position=(0, 0))
                            nc.tensor.matmul(sc[:, 2 * QC:3 * QC],
                                             r(kt0[64:128, :]), qB,
                                             start=True, stop=False,
                                             tile_position=(64, 0))
                            nc.tensor.matmul(sc[:, QC:2 * QC], r(kt1[0:64, :]),
                                             qA, start=False, stop=not masked,
                                             tile_position=(0, 0))
                            nc.tensor.matmul(sc[:, 3 * QC:4 * QC],
                                             r(kt1[64:128, :]), qB,
                                             start=False, stop=not masked,
                                             tile_position=(64, 0))
                            if masked:
                                li = (k * 4 + (j0 - (PN[k] - 4))) * QC
                                mb = masks_sb[:, li:li + 2 * QC]
                                nc.tensor.matmul(sc[:, 0:2 * QC], eye_sb[:], mb,
                                                 start=False, stop=True)
                                nc.tensor.matmul(sc[:, 2 * QC:4 * QC], eye_sb[:],
                                                 mb, start=False, stop=True)
                            e_t = epool.tile([P, 4 * QC], F32R, tag="exp")
                            nc.scalar.activation(e_t[:], sc[:], EXP, scale=SCALE)
                            if pend is not None:
                                emit_av(pend)
                            pend = (e_t, v0, v1, j0)
                    emit_av(pend)
                    rs = npool.tile([1, 2 * QC], F32, tag="rs", bufs=2)
                    nc.vector.tensor_copy(rs[:], avp[64:65, :])
                    avc = npool.tile([64, 2 * QC], F32, tag="avc", bufs=2)
                    nc.vector.tensor_copy(avc[:], avp[0:64, :])
                    rc = npool.tile([1, 2 * QC], F32, tag="rc", bufs=2)
                    nc.vector.reciprocal_approx_fast(rc[:], rs[:])
                    rb = npool.tile([64, 2 * QC], F32, tag="rb", bufs=2)
                    nc.gpsimd.partition_broadcast(rb[:], rc[:])
                    col = p * 1024 + k * QC
                    nc.vector.tensor_mul(attn_sb[0:64, col:col + QC],
                                         avc[:, 0:QC], rb[:, 0:QC])
                    nc.vector.tensor_mul(attn_sb[64:128, col:col + QC],
                                         avc[:, QC:2 * QC], rb[:, QC:2 * QC])

                def proj_unit(k, tt, oc):
                    pp = ps_pj.tile([P, 512], F32, tag="pj")
                    for g in range(NP):
                        nc.tensor.matmul(
                            pp[:],
                            r(attn_sb[:, g * 1024 + k * QC + tt * P:][:, :P]),
                            wp_sb[:, g * 1024 + oc * 512:][:, :512],
                            start=(g == 0), stop=(g == 7),
                        )
                    ot = outpool.tile([P, 512], F32, tag="ot")
                    nc.vector.tensor_add(
                        ot[:], pp[:], bias_bc[:, oc * 512:(oc + 1) * 512]
                    )
                    nc.sync.dma_start(
                        out[k, tt * P:(tt + 1) * P, oc * 512:(oc + 1) * 512],
                        ot[:],
                    )

                for k in (3, 2, 1, 0):
                    for p in range(NP):
                        c_run(k, p)
                    for tt in range(2):
                        for oc in range(2):
                            proj_unit(k, tt, oc)
    return nc


def _make_masks(half):
    chunks = CHUNKS[half]
    m = np.zeros((16, P, QC), np.float32)
    s = np.arange(P)[:, None]
    t = np.arange(QC)[None, :]
    for k in range(4):
        q = chunks[k]
        n = 2 * (q + 1)
        for l in range(4):
            j = PN[k] - 4 + l
            if j >= n:
                pat = np.full((P, QC), -1e6, np.float32)
            elif j == n - 2:
                pat = np.where(s <= t, 0.0, -1e6).astype(np.float32)
            elif j == n - 1:
                pat = np.where(s <= t - 128, 0.0, -1e6).astype(np.float32)
            else:
                pat = np.zeros((P, QC), np.float32)
            m[k * 4 + l] = pat
    return m


_CACHE = {}


def _get_nc():
    if "nc" not in _CACHE:
        nc = bacc.Bacc("TRN2", target_bir_lowering=False, debug=False)
        build_kernel(nc)
        nc.compile()
        _CACHE["nc"] = nc
    return _CACHE["nc"]


def make_in_maps(x, wq, wk, wv, w_proj, b_proj):
    x = np.ascontiguousarray(np.asarray(x, np.float32))
    wq2 = np.ascontiguousarray(np.transpose(np.asarray(wq), (1, 0, 2)).reshape(C, C))
    wk2 = np.ascontiguousarray(np.transpose(np.asarray(wk), (1, 0, 2)).reshape(C, C))
    wv2 = np.ascontiguousarray(np.transpose(np.asarray(wv), (1, 0, 2)).reshape(C, C))
    wpm = np.ascontiguousarray(np.asarray(w_proj, np.float32))
    bias = np.asarray(b_proj, np.float32).reshape(1, C)
    masks_h = [_make_masks(0), _make_masks(1)]

    in_maps = []
    for core in range(8):
        b, half = core // 2, core % 2
        xTb = np.ascontiguousarray(x[b].T)
        xqb = np.ascontiguousarray(
            np.concatenate(
                [xTb[:, q * QC:(q + 1) * QC] for q in CHUNKS[half]], axis=1
            )
        )
        in_maps.append({
            "xT": xTb, "xq": xqb,
            "wq2": wq2, "wk2": wk2, "wv2": wv2,
            "wp": wpm, "bias": bias, "masks": masks_h[half],
            "eye": np.eye(P, dtype=np.float32),
        })
    return in_maps


def assemble(results):
    full = np.zeros((B, T, C), np.float32)
    for core in range(8):
        b, half = core // 2, core % 2
        o = results[core]["out"]
        for k, q in enumerate(CHUNKS[half]):
            full[b, q * QC:(q + 1) * QC] = o[k]
    return full


def kernel(x, wq, wk, wv, w_proj, b_proj, _trace=False, _tmpdir=None):
    in_maps = make_in_maps(x, wq, wk, wv, w_proj, b_proj)
    nc = _get_nc()
    res = run_bass_kernel_spmd(
        nc, in_maps, core_ids=list(range(8)), trace=_trace, tmpdir=_tmpdir
    )
    if _trace:
        _CACHE["last_result"] = res
    return assemble(res.results)



# revision 8
# speedup vs baseline: 1.6438x; 1.6438x over previous
"""Multi-head causal attention (B=4,T=2048,C=1024,H=16,D=64) on 8 TRN2 NeuronCores.

Sharding: no collectives. Core c handles batch b=c//2 and a causally-balanced
set of four 256-query chunks (half=c%2): half0 -> chunks [0,2,5,7], half1 ->
[1,3,4,6], processed in slot order with padded per-slot key-tile counts
[4,8,12,16]. Every core runs the same SPMD program; per-core differences are
expressed purely through input data:
  - half1 cores get xT with the two 256-token blocks of each 512-token stripe
    swapped, which makes the query-column offset of the slot-s chunk inside
    stripe s uniform ([0,0,256,256]) across halves, and
  - per-half multiplicative {0,1} causal/pad masks for the last 4 key tiles of
    each slot (key order follows the same permutation; attention is
    key-order-invariant).

All-bf16 (inputs converted on host), K/V SBUF-resident (no DRAM scratch),
causal masks as 0/1 DVE multiplies, x streamed per 512-token stripe.

Schedule (one Bass/Tile program): the C-stage of slot s-1 is interleaved
between the K-projection groups of stripe s, so the in-order PE queue always
has independent projection matmuls between exp-dependent score/AV matmuls:
  s=0: x0 DMA, Q(0), K(0), V(0)
  s>0: xs DMA, Q(s), [C(s-1,p) | K(s,p) for p in 0..7], V(s), proj(s-2)
  tail: C(3,p) for p, proj(2) interleaved, proj(3)
C(slot): per head-pair p, per key-tile pair: scores = kT^T @ qT (two heads
packed via PE row groups, separate PSUM banks), exp on ACT (scale folded),
0/1 mask multiplies on DVE for the last 4 key tiles, wei @ [v|1] accumulated
in PSUM ([65,512] per pair: row 64 = sumexp; one start/stop per bank).
Normalization is decoupled: avp is staged to SBUF bf16 (frees the PSUM slot),
then DVE fast reciprocal + GPSIMD partition broadcast + DVE multiplies.
"""

import numpy as np
import ml_dtypes

import concourse.bass as bass
import concourse.tile as tile
from concourse import bacc, library_config, mybir
from concourse.bass_utils import run_bass_kernel_spmd

B, T, C = 4, 2048, 1024
H, D = 16, 64
P = 128            # key tile size
QC = 256           # query chunk size
NP = 8             # head pairs
PNS = [4, 8, 12, 16]                     # padded per-slot key-tile counts
SLOT_CHUNKS = [[0, 2, 5, 7], [1, 3, 4, 6]]  # chunk ids per half, slot order
QOFF = [0, 0, 256, 256]  # query-col offset of slot-s chunk inside stripe s
BF16 = mybir.dt.bfloat16
F32 = mybir.dt.float32
EXP = mybir.ActivationFunctionType.Exp
SCALE = float(C) ** -0.5
VW = 130           # v cols per pair: [vA(64) | 1 | vB(64) | 1]
BD = ml_dtypes.bfloat16


def build_kernel(nc: bass.Bass):
    xT = nc.dram_tensor("xT", [C, T], BF16, kind="ExternalInput").ap()
    wq2 = nc.dram_tensor("wq2", [C, C], BF16, kind="ExternalInput").ap()
    wk2 = nc.dram_tensor("wk2", [C, C], BF16, kind="ExternalInput").ap()
    wv2 = nc.dram_tensor("wv2", [C, C], BF16, kind="ExternalInput").ap()
    wp = nc.dram_tensor("wp", [C, C], BF16, kind="ExternalInput").ap()
    bias2 = nc.dram_tensor("bias2", [P, C], BF16, kind="ExternalInput").ap()
    masks = nc.dram_tensor("masks", [P, 16 * QC], BF16, kind="ExternalInput").ap()
    out = nc.dram_tensor("out", [4, QC, C], BF16, kind="ExternalOutput").ap()

    with tile.TileContext(nc) as tc:
        nc.gpsimd.load_library(library_config.attn)
        with (
            tc.tile_pool(name="const", bufs=1) as cpool,
            tc.tile_pool(name="xs", bufs=2) as xpool,
            tc.tile_pool(name="exp", bufs=3) as epool,
            tc.tile_pool(name="outp", bufs=2) as opool,
            tc.tile_pool(name="norm", bufs=1) as npool,
            tc.tile_pool(name="ps", bufs=2, space="PSUM") as psp,
        ):
            wq_sb = cpool.tile([P, 8 * C], BF16)
            wk_sb = cpool.tile([P, 8 * C], BF16)
            wv_sb = cpool.tile([P, 8 * C], BF16)
            wp_sb = cpool.tile([P, 8 * C], BF16)
            qT_sb = cpool.tile([P, NP * 1024], BF16)
            kT_sb = cpool.tile([P, NP * T], BF16)
            v_sb = cpool.tile([P, 16 * NP * VW], BF16)
            attn_sb = cpool.tile([P, NP * 1024], BF16)
            masks_sb = cpool.tile([P, 16 * QC], BF16)
            bias_bc = cpool.tile([P, C], BF16)

            def dma_w(dst, src):
                # whole [C, C] weight -> [128, 8*C] SBUF in one DMA
                nc.sync.dma_start(
                    dst[:].rearrange("p (g c) -> p g c", c=C),
                    src.rearrange("(g p) c -> p g c", p=P),
                )

            def dma_x(xs, s):
                nc.sync.dma_start(
                    xs[:].rearrange("p (g c) -> p g c", c=512),
                    xT.rearrange("(g p) t -> p g t", p=P)[
                        :, :, s * 512:(s + 1) * 512],
                )

            def q_stage(s, xs):
                for p in range(NP):
                    qp = psp.tile([P, QC], F32, tag="mm", name=f"qp{s}_{p}")
                    for g in range(8):
                        nc.tensor.matmul(
                            qp[:],
                            wq_sb[:, g * C + p * P:][:, :P],
                            xs[:, g * 512 + QOFF[s]:][:, :QC],
                            start=(g == 0), stop=(g == 7),
                        )
                    nc.scalar.copy(qT_sb[:, p * 1024 + s * QC:][:, :QC], qp[:])

            def k_group(s, xs, p):
                kp = psp.tile([P, 512], F32, tag="mm", name=f"kp{s}_{p}")
                for g in range(8):
                    nc.tensor.matmul(
                        kp[:],
                        wk_sb[:, g * C + p * P:][:, :P],
                        xs[:, g * 512:(g + 1) * 512],
                        start=(g == 0), stop=(g == 7),
                    )
                nc.vector.tensor_copy(kT_sb[:, p * T + s * 512:][:, :512], kp[:])

            def v_stage(s, xs):
                for jj in range(4):
                    j = 4 * s + jj
                    for hc in range(2):
                        vp = psp.tile([P, 512], F32, tag="mm",
                                      name=f"vp{j}_{hc}")
                        for g in range(8):
                            nc.tensor.matmul(
                                vp[:],
                                xs[:, g * 512 + jj * P:][:, :P],
                                wv_sb[:, g * C + hc * 512:][:, :512],
                                start=(g == 0), stop=(g == 7),
                            )
                        vdst = v_sb[:, j * (NP * VW) + hc * 4 * VW:][:, :4 * VW]
                        v3 = vdst.rearrange("p (l c) -> p l c", c=VW)
                        s3 = vp[:].rearrange("p (l c) -> p l c", c=P)
                        nc.scalar.copy(v3[:, :, 0:64], s3[:, :, 0:64])
                        nc.scalar.copy(v3[:, :, 65:129], s3[:, :, 64:128])

            def c_run(k, p):
                pn = PNS[k]
                avp = psp.tile([65, 512], F32, tag="av", name=f"av{k}_{p}")
                qA = qT_sb[0:64, p * 1024 + k * QC:][:, :QC]
                qB = qT_sb[64:128, p * 1024 + k * QC:][:, :QC]
                pend = None

                def emit_av(pv):
                    # avp is ONE 2KB PSUM bank: exactly one start=True (bank
                    # clear) on the first matmul and one stop=True on the
                    # last; interior matmuls overwrite-where-unset.
                    e_t, j0 = pv
                    last = j0 + 2 == pn
                    b0 = j0 * (NP * VW) + p * VW
                    b1 = (j0 + 1) * (NP * VW) + p * VW
                    nc.tensor.matmul(avp[:, 0:QC], v_sb[:, b0:b0 + 65],
                                     e_t[:, 0:QC],
                                     start=(j0 == 0), stop=False)
                    nc.tensor.matmul(avp[:, 0:QC], v_sb[:, b1:b1 + 65],
                                     e_t[:, QC:2 * QC],
                                     start=False, stop=False)
                    nc.tensor.matmul(avp[:, QC:2 * QC], v_sb[:, b0 + 65:b0 + VW],
                                     e_t[:, 2 * QC:3 * QC],
                                     start=False, stop=False)
                    nc.tensor.matmul(avp[:, QC:2 * QC], v_sb[:, b1 + 65:b1 + VW],
                                     e_t[:, 3 * QC:4 * QC],
                                     start=False, stop=last)

                for u in range(pn // 2):
                    j0 = 2 * u
                    kt0 = kT_sb[:, p * T + j0 * P:][:, :P]
                    kt1 = kT_sb[:, p * T + (j0 + 1) * P:][:, :P]
                    # sc spans 2 PSUM banks (cols 0:512 / 512:1024): one
                    # start=True per bank; the second matmul into a bank
                    # runs accumulate-mode and overwrites its untouched half.
                    sc = psp.tile([P, 4 * QC], F32, tag="sc",
                                  name=f"sc{k}_{p}_{u}")
                    nc.tensor.matmul(sc[:, 0:QC], kt0[0:64, :], qA,
                                     start=True, stop=False,
                                     tile_position=(0, 0))
                    nc.tensor.matmul(sc[:, 2 * QC:3 * QC], kt0[64:128, :], qB,
                                     start=True, stop=False,
                                     tile_position=(64, 0))
                    nc.tensor.matmul(sc[:, QC:2 * QC], kt1[0:64, :], qA,
                                     start=False, stop=True,
                                     tile_position=(0, 0))
                    nc.tensor.matmul(sc[:, 3 * QC:4 * QC], kt1[64:128, :], qB,
                                     start=False, stop=True,
                                     tile_position=(64, 0))
                    e_t = epool.tile([P, 4 * QC], BF16, tag="e",
                                     name=f"e{k}_{p}_{u}")
                    nc.scalar.activation(e_t[:], sc[:], EXP, scale=SCALE)
                    if u >= pn // 2 - 2:
                        l0 = j0 - (pn - 4)
                        mi = (k * 4 + l0) * QC
                        m2 = masks_sb[:, mi:mi + 2 * QC]
                        nc.vector.tensor_mul(e_t[:, 0:2 * QC],
                                             e_t[:, 0:2 * QC], m2)
                        nc.vector.tensor_mul(e_t[:, 2 * QC:4 * QC],
                                             e_t[:, 2 * QC:4 * QC], m2)
                    if pend is not None:
                        emit_av(pend)
                    pend = (e_t, j0)
                emit_av(pend)

                # stage avp to SBUF (frees the PSUM slot), then normalize
                avst = npool.tile([65, 512], BF16, tag="avst", bufs=2,
                                  name=f"avst{k}_{p}")
                nc.vector.tensor_copy(avst[:], avp[:])
                rs = npool.tile([1, 2 * QC], F32, tag="rs", name=f"rs{k}_{p}")
                nc.vector.tensor_copy(rs[:], avst[64:65, :])
                rc = npool.tile([1, 2 * QC], F32, tag="rc", name=f"rc{k}_{p}")
                nc.vector.reciprocal_approx_fast(rc[:], rs[:])
                rb = npool.tile([64, 2 * QC], F32, tag="rb", name=f"rb{k}_{p}")
                nc.gpsimd.partition_broadcast(rb[:], rc[:])
                col = p * 1024 + k * QC
                nc.vector.tensor_mul(attn_sb[0:64, col:col + QC],
                                     avst[0:64, 0:QC], rb[:, 0:QC])
                nc.vector.tensor_mul(attn_sb[64:128, col:col + QC],
                                     avst[0:64, QC:2 * QC], rb[:, QC:2 * QC])

            def proj(k):
                for tt in range(2):
                    for oc in range(2):
                        pp = psp.tile([P, 512], F32, tag="mm",
                                      name=f"pp{k}_{tt}_{oc}")
                        for g in range(NP):
                            nc.tensor.matmul(
                                pp[:],
                                attn_sb[:, g * 1024 + k * QC + tt * P:][:, :P],
                                wp_sb[:, g * C + oc * 512:][:, :512],
                                start=(g == 0), stop=(g == 7),
                            )
                        ot = opool.tile([P, 512], BF16, tag="ot",
                                        name=f"ot{k}_{tt}_{oc}")
                        nc.vector.tensor_add(
                            ot[:], pp[:], bias_bc[:, oc * 512:(oc + 1) * 512]
                        )
                        nc.sync.dma_start(
                            out[k, tt * P:(tt + 1) * P, oc * 512:(oc + 1) * 512],
                            ot[:],
                        )

            # startup: interleave x-stripe-0 and wq per-g DMAs so Q(0) can
            # begin after the first blocks land; bulk weights follow.
            xs0 = xpool.tile([P, 8 * 512], BF16, tag="xs", name="xs0")
            for g in range(8):
                nc.sync.dma_start(
                    xs0[:, g * 512:(g + 1) * 512],
                    xT[g * P:(g + 1) * P, 0:512],
                )
                nc.sync.dma_start(
                    wq_sb[:, g * C:(g + 1) * C], wq2[g * P:(g + 1) * P, :]
                )
            dma_w(wk_sb, wk2)
            dma_w(wv_sb, wv2)
            q_stage(0, xs0)
            for p in range(NP):
                k_group(0, xs0, p)
            dma_w(wp_sb, wp)
            nc.sync.dma_start(masks_sb[:], masks[:])
            nc.sync.dma_start(bias_bc[:], bias2[:])
            # ones columns of v (col = 65*m + 64 for m in 0..255)
            vones = v_sb[:].rearrange("p (m o) -> p m o", o=65)[:, :, 64:65]
            nc.vector.memset(vones, 1.0)
            v_stage(0, xs0)

            for s in range(1, 4):
                xs = xpool.tile([P, 8 * 512], BF16, tag="xs", name=f"xs{s}")
                dma_x(xs, s)
                q_stage(s, xs)
                for p in range(NP):
                    c_run(s - 1, p)
                    k_group(s, xs, p)
                v_stage(s, xs)
                if s >= 2:
                    proj(s - 2)
            for p in range(NP):
                c_run(3, p)
                if p == 4:
                    proj(2)
            proj(3)
    return nc


def _make_masks(half):
    m = np.zeros((P, 16 * QC), np.float32)
    s = np.arange(P)[:, None]
    t = np.arange(QC)[None, :]
    for k in range(4):
        q = SLOT_CHUNKS[half][k]
        pn = PNS[k]
        for l in range(4):
            j = pn - 4 + l
            a = j if half == 0 else 4 * (j // 4) + (j + 2) % 4
            m[:, (k * 4 + l) * QC:(k * 4 + l + 1) * QC] = (
                a * P + s <= q * QC + t
            )
    return m.astype(BD)


_CACHE = {}


def _get_nc():
    if "nc" not in _CACHE:
        nc = bacc.Bacc("TRN2", target_bir_lowering=False, debug=False)
        build_kernel(nc)
        nc.compile()
        _CACHE["nc"] = nc
    return _CACHE["nc"]


def make_in_maps(x, wq, wk, wv, w_proj, b_proj):
    x = np.asarray(x, np.float32)
    wq2 = np.ascontiguousarray(
        np.transpose(np.asarray(wq), (1, 0, 2)).reshape(C, C)).astype(BD)
    wk2 = np.ascontiguousarray(
        np.transpose(np.asarray(wk), (1, 0, 2)).reshape(C, C)).astype(BD)
    wv2 = np.ascontiguousarray(
        np.transpose(np.asarray(wv), (1, 0, 2)).reshape(C, C)).astype(BD)
    wpm = np.asarray(w_proj, np.float32).astype(BD)
    bias2 = np.tile(np.asarray(b_proj, np.float32).reshape(1, C), (P, 1))
    bias2 = np.ascontiguousarray(bias2).astype(BD)
    masks_h = [_make_masks(0), _make_masks(1)]

    in_maps = []
    for core in range(8):
        b, half = core // 2, core % 2
        xb = x[b]
        if half == 1:
            # swap the two 256-blocks of each 512-token stripe
            xb = xb.reshape(4, 2, QC, C)[:, ::-1].reshape(T, C)
        xTb = np.ascontiguousarray(xb.T).astype(BD)
        in_maps.append({
            "xT": xTb,
            "wq2": wq2, "wk2": wk2, "wv2": wv2,
            "wp": wpm, "bias2": bias2, "masks": masks_h[half],
        })
    return in_maps


def assemble(results):
    full = np.zeros((B, T, C), np.float32)
    for core in range(8):
        b, half = core // 2, core % 2
        o = np.asarray(results[core]["out"], dtype=np.float32)
        for k, q in enumerate(SLOT_CHUNKS[half]):
            full[b, q * QC:(q + 1) * QC] = o[k]
    return full


def kernel(x, wq, wk, wv, w_proj, b_proj, _trace=False, _tmpdir=None):
    in_maps = make_in_maps(x, wq, wk, wv, w_proj, b_proj)
    nc = _get_nc()
    res = run_bass_kernel_spmd(
        nc, in_maps, core_ids=list(range(8)), trace=_trace, tmpdir=_tmpdir
    )
    if _trace:
        _CACHE["last_result"] = res
    return assemble(res.results)
